# revision 1
# baseline (speedup 1.0000x reference)
"""Trainium2 Bass kernel for nn_BaselineMemory (sparse attention memory read + MLP).

Data-parallel over batch: each of 8 NeuronCores handles 256 of 2048 rows.
Pipeline per core:
  x-norm (ACT) -> dist matmul z = x_hat @ y_hat^T - 1 (fp32r, PE)
  -> sparsemax via log-secant threshold iteration (ACT relu+bias+accum on the
     head of m, DVE chunked max+sum on the tail: S = sum max(z,tau) - n*tau)
  -> w^T transpose (PE) -> memory read mv^T (bf16, PE)
  -> MLP1 (bf16, transposed layout; b1 fused as per-partition ACT bias + ReLU)
  -> MLP2 (bf16; b2 via rank-1 fp32r matmul) -> fp32 out.
"""
import sys

if "/opt/trn_rl_repo" not in sys.path:
    sys.path.insert(0, "/opt/trn_rl_repo")

import numpy as np
import ml_dtypes

import concourse.bass as bass  # noqa: F401
import concourse.tile as tile
from concourse import bacc, mybir
from concourse.bass_utils import run_bass_kernel_spmd
from concourse.masks import make_identity

P = 128
B_CORE = 256          # batch rows per core
NBT = B_CORE // P     # 2 b-tiles
D = 1024
DC = D // P           # 8 d-chunks
M = 8192
MC512 = M // 512      # 16 m-chunks for dist
MC128 = M // P        # 64 m-chunks for read
H = 2048
HC = H // P           # 16 h-chunks
OUT = 1000
NH = 2                # out halves of 500
N_SECANT = 7          # secant iterations after the init pass
MA = 6144             # ACT handles m [0, MA); DVE chunks handle [MA, M)
NDV = (M - MA) // 512  # 7 DVE chunks of 512

F32 = mybir.dt.float32
F32R = mybir.dt.float32r
BF16 = mybir.dt.bfloat16
AF = mybir.ActivationFunctionType
ALU = mybir.AluOpType
AX = mybir.AxisListType
bf16 = ml_dtypes.bfloat16

_EPS = 1e-6


def build():
    nc = bacc.Bacc("TRN2", target_bir_lowering=False, debug=False)

    x_s = nc.dram_tensor("x_s", [NBT, P, D], F32, kind="ExternalInput")
    memT = nc.dram_tensor("memT", [DC, P, M], F32R, kind="ExternalInput")
    mem_bf = nc.dram_tensor("mem_bf", [MC128, P, D], BF16, kind="ExternalInput")
    # host-prepped partition-major layouts (contiguous per-partition runs)
    w1_bf = nc.dram_tensor("w1_bf", [P, DC, HC, P], BF16, kind="ExternalInput")
    w2_bf = nc.dram_tensor("w2_bf", [P, HC, OUT], BF16, kind="ExternalInput")
    b1_t = nc.dram_tensor("b1_t", [P, HC], F32, kind="ExternalInput")
    b2_r = nc.dram_tensor("b2_r", [1, OUT], F32R, kind="ExternalInput")
    out_d = nc.dram_tensor("out", [NBT, P, OUT], F32, kind="ExternalOutput")

    with tile.TileContext(nc) as tc:
        small = tc.alloc_tile_pool(name="small", bufs=1)
        pers = tc.alloc_tile_pool(name="pers", bufs=1)

        ident = small.tile([P, P], F32, tag="ident")
        make_identity(nc, ident[:])
        eps_t = small.tile([P, 1], F32, tag="eps")
        nc.vector.memset(eps_t[:], _EPS)
        b1t = small.tile([P, HC], F32, tag="b1")
        nc.sync.dma_start(b1t[:], b1_t[:])
        b2t = small.tile([1, OUT], F32R, tag="b2")
        nc.sync.dma_start(b2t[:], b2_r[:])
        ones1f = small.tile([1, P], F32, tag="ones1f")
        nc.vector.memset(ones1f[:], 1.0)
        ones1 = small.tile([1, P], F32R, tag="ones1")
        nc.vector.tensor_copy(ones1[:], ones1f[:])

        # ---- x load + normalize + transpose -> xnT [P, dc, 256] fp32r ----
        xnT = pers.tile([P, DC, B_CORE], F32R, tag="xnT")
        xpool = tc.alloc_tile_pool(name="xpool", bufs=1)
        ps_x = tc.alloc_tile_pool(name="ps_x", bufs=2, space="PSUM")
        xn = []
        inv_x = []
        for bt in range(NBT):
            xt = xpool.tile([P, D], F32, tag=f"x{bt}")
            nc.sync.dma_start(xt[:], x_s[bt])
            ss = small.tile([P, 1], F32, tag=f"ss{bt}")
            sq = xpool.tile([P, D], F32, tag="sqscratch")
            nc.scalar.activation(sq[:], xt[:], AF.Square, accum_out=ss[:])
            nrm = small.tile([P, 1], F32, tag=f"nrm{bt}")
            nc.scalar.activation(nrm[:], ss[:], AF.Sqrt, bias=eps_t[:, 0:1])
            inv = small.tile([P, 1], F32, tag=f"inv{bt}")
            nc.vector.reciprocal(inv[:], nrm[:])
            inv_x.append(inv)
            xn.append(xt)
        # transpose RAW x (row scaling folded into the z evacuation; sparsemax
        # is shift-invariant so the -1 is dropped entirely: z = cos)
        for dc in range(DC):
            pt = ps_x.tile([P, B_CORE], F32, tag="xtr")
            for bt in range(NBT):
                nc.tensor.transpose(
                    pt[:, bt * P:(bt + 1) * P],
                    xn[bt][:, dc * P:(dc + 1) * P], ident[:])
            nc.vector.tensor_copy(xnT[:, dc], pt[:])
        ps_x.release()
        xpool.release()

        # Slot-sharing pools: wpool tags w0/w1 (32KB slots), zpool tags z0/z1.
        wpool = tc.alloc_tile_pool(name="wpool", bufs=1)
        w = [wpool.tile([P, M], F32, tag=f"w{bt}", name=f"w{bt}") for bt in range(NBT)]
        zpool = tc.alloc_tile_pool(name="zpool", bufs=1)
        z = [zpool.tile([P, M], F32, tag=f"z{bt}", name=f"z{bt}") for bt in range(NBT)]
        mstream = tc.alloc_tile_pool(name="mstream", bufs=2)

        # ---- dist matmul: z[bt] [P, M] fp32 (= cos - 1) + chunk maxes + sums ----
        mx = [small.tile([P, MC512], F32, tag=f"mx{bt}", name=f"mx{bt}")
              for bt in range(NBT)]
        zsum = [small.tile([P, MC512], F32, tag=f"zs{bt}", name=f"zs{bt}")
                for bt in range(NBT)]
        ps_dist = tc.alloc_tile_pool(name="ps_dist", bufs=6, space="PSUM")
        for mc in range(MC512):
            mtile = mstream.tile([P, DC, 512], F32R, tag="memT")
            for dq in range(4):
                nc.sync.dma_start(
                    mtile[:, dq * 2:(dq + 1) * 2],
                    memT[dq * 2:(dq + 1) * 2, :, mc * 512:(mc + 1) * 512]
                    .rearrange("d p m -> p d m"))
            for bt in range(NBT):
                zp = ps_dist.tile([P, 512], F32, tag="zp")
                for dc in range(DC):
                    nc.tensor.matmul(
                        zp[:], xnT[:, dc, bt * P:(bt + 1) * P], mtile[:, dc],
                        start=(dc == 0), stop=(dc == DC - 1))
                nc.vector.tensor_scalar(
                    out=z[bt][:, mc * 512:(mc + 1) * 512], in0=zp[:],
                    scalar1=inv_x[bt][:, 0:1], scalar2=None,
                    op0=ALU.mult, op1=ALU.add,
                    accum_out=zsum[bt][:, mc:mc + 1])
                nc.vector.reduce_max(
                    mx[bt][:, mc:mc + 1], zp[:], axis=AX.X)
        ps_dist.release()

        # ---- sparsemax via log-secant; S(tau) = ACT head + DVE tail chunks ----
        ps_warm = tc.alloc_tile_pool(name="ps_warm", bufs=2, space="PSUM")
        CAP_OFF = 1e-4

        tail_scr = [small.tile([P, M - MA], F32, tag=f"tailscr{b}", name=f"ts{b}")
                    for b in range(NBT)]

        def s_pass(bt, stt):
            tau_c, ntau = stt["tau_c"], stt["ntau"]
            s_act, gacc, s_v = stt["s_act"], stt["gacc"], stt["s_v"]
            nc.scalar.activation(
                w[bt][:, 0:MA], z[bt][:, 0:MA], AF.Relu,
                bias=ntau[:, 0:1], accum_out=s_act[:])
            # relu in two DVE ops: max into scratch, then (x - tau) with
            # fused add-reduce accum (sums small positives -> no cancellation)
            nc.vector.tensor_scalar(
                out=tail_scr[bt][:], in0=z[bt][:, MA:M],
                scalar1=tau_c[:, 0:1], scalar2=None, op0=ALU.max)
            nc.vector.tensor_scalar(
                out=w[bt][:, MA:M], in0=tail_scr[bt][:],
                scalar1=tau_c[:, 0:1], scalar2=None,
                op0=ALU.subtract, op1=ALU.add, accum_out=gacc[:, 0:1])
            nc.vector.tensor_add(s_v[:], gacc[:, 0:1], s_act[:])
            # PE warmers: keep HAM at 8/8 through the sparsemax window
            wp = ps_warm.tile([P, 512], F32, tag="warm")
            nc.tensor.matmul(wp[:], ident[:], w[bt][:, 0:512],
                             start=True, stop=True)
            nc.tensor.matmul(wp[:], ident[:], w[bt][:, 512:1024],
                             start=True, stop=True)

        st = {}
        for bt in range(NBT):
            stt = {}
            for nm in ["rm", "cap", "tau_p", "tau_c", "l_p", "l_c", "ntau",
                       "s_v", "s_act", "gs", "corr", "dl", "rdl", "dt",
                       "step", "neg"]:
                stt[nm] = small.tile([P, 1], F32, tag=f"{nm}{bt}", name=f"{nm}{bt}")
            stt["gacc"] = small.tile([P, NDV], F32, tag=f"gacc{bt}", name=f"gacc{bt}")
            st[bt] = stt
            rm, cap, tau_p, tau_c = stt["rm"], stt["cap"], stt["tau_p"], stt["tau_c"]
            l_p, ntau, s_v = stt["l_p"], stt["ntau"], stt["s_v"]
            nc.vector.reduce_max(rm[:], mx[bt][:], axis=AX.X)
            nc.vector.tensor_mul(rm[:], rm[:], inv_x[bt][:])  # rowmax of z=cos
            nc.vector.tensor_scalar_add(cap[:], rm[:], -CAP_OFF)
            nc.vector.tensor_scalar_add(tau_p[:], rm[:], -1.0)
            # analytic S0 = sum(z) - M*tau_p (tau_p = rowmax-1; <= true S, safe)
            zs = stt["gs"]
            nc.vector.reduce_sum(zs[:], zsum[bt][:], axis=AX.X)
            nc.vector.tensor_scalar_mul(s_v[:], tau_p[:], -float(M))
            nc.vector.tensor_add(s_v[:], s_v[:], zs[:])
            nc.vector.tensor_scalar_max(s_v[:], s_v[:], 1.0)  # guard ln<=0
            nc.scalar.activation(l_p[:], s_v[:], AF.Ln)
            # tau_c = tau_p + (S0-1)/M, capped
            nc.vector.tensor_scalar(
                out=tau_c[:], in0=s_v[:], scalar1=-1.0, scalar2=1.0 / M,
                op0=ALU.add, op1=ALU.mult)
            nc.vector.tensor_add(tau_c[:], tau_c[:], tau_p[:])
            nc.vector.tensor_tensor(tau_c[:], tau_c[:], cap[:], ALU.min)
            nc.vector.tensor_scalar_mul(ntau[:], tau_c[:], -1.0)
            s_pass(bt, stt)
            nc.scalar.activation(stt["l_c"][:], s_v[:], AF.Ln)

        for it in range(N_SECANT):
            for bt in range(NBT):
                stt = st[bt]
                cap, tau_p, tau_c = stt["cap"], stt["tau_p"], stt["tau_c"]
                l_p, l_c, ntau = stt["l_p"], stt["l_c"], stt["ntau"]
                dl, rdl, dt = stt["dl"], stt["rdl"], stt["dt"]
                step = stt["step"]
                nc.vector.tensor_sub(dl[:], l_p[:], l_c[:])
                nc.vector.tensor_scalar(
                    out=dl[:], in0=dl[:], scalar1=-1.0, scalar2=dl[:, 0:1],
                    op0=ALU.mult, op1=ALU.max)   # |dl|
                nc.vector.tensor_scalar_max(dl[:], dl[:], 1e-12)
                nc.vector.reciprocal(rdl[:], dl[:])
                nc.vector.tensor_sub(dt[:], tau_c[:], tau_p[:])
                nc.vector.tensor_scalar(
                    out=dt[:], in0=dt[:], scalar1=-1.0, scalar2=dt[:, 0:1],
                    op0=ALU.mult, op1=ALU.max)   # |dt|
                nc.vector.tensor_scalar(
                    out=step[:], in0=dt[:], scalar1=rdl[:, 0:1], scalar2=2.0,
                    op0=ALU.mult, op1=ALU.min)   # q = min(|dt|/|dl|, 2)
                nc.vector.tensor_copy(tau_p[:], tau_c[:])
                nc.vector.tensor_mul(step[:], step[:], l_c[:])
                nc.vector.tensor_scalar(
                    out=tau_c[:], in0=step[:], scalar1=tau_c[:, 0:1],
                    scalar2=cap[:, 0:1], op0=ALU.add, op1=ALU.min)
                nc.vector.tensor_copy(l_p[:], l_c[:])
                nc.vector.tensor_scalar_mul(ntau[:], tau_c[:], -1.0)
                s_pass(bt, stt)  # last iteration leaves w (head exact relu)
            if it != N_SECANT - 1:
                # Lns emitted after BOTH passes: avoids ACT FIFO head-of-line
                # blocking (Ln waits on the DVE tail sum; a pass queued behind
                # it would stall)
                for bt in range(NBT):
                    nc.scalar.activation(st[bt]["l_c"][:], st[bt]["s_v"][:], AF.Ln)
        ps_warm.release()

        # prefetch first mem slab during the sparsemax/transpose window
        mtile0 = mstream.tile([P, 4, D], BF16, tag="memT", name="membf0")
        for dq in range(2):
            nc.sync.dma_start(
                mtile0[:, dq * 2:(dq + 1) * 2],
                mem_bf[dq * 2:(dq + 1) * 2].rearrange("c p d -> p c d"))

        # ---- wT transposes interleaved with read matmuls (per-mc pipeline) ----
        wTt = zpool.tile([P, MC128, B_CORE], BF16, tag="z0", name="wTt")
        w1t = zpool.tile([P, DC, HC, P], BF16, tag="z1", name="w1t")
        for dq in range(4):
            nc.sync.dma_start(w1t[:, dq * 2:(dq + 1) * 2],
                              w1_bf[:, dq * 2:(dq + 1) * 2])
        ps_trw = tc.alloc_tile_pool(name="ps_trw", bufs=4, space="PSUM")
        ps_mv = tc.alloc_tile_pool(name="ps_mv", bufs=1, space="PSUM")
        mv_ps = [[ps_mv.tile([P, 512], F32, tag=f"mv{bt}_{dh}", name=f"mv{bt}_{dh}")
                  for dh in range(2)] for bt in range(NBT)]
        for mc4 in range(MC128 // 4):
            if mc4 == 0:
                mtile = mtile0
            else:
                mtile = mstream.tile([P, 4, D], BF16, tag="memT", name="membf")
                for dq in range(2):
                    nc.sync.dma_start(
                        mtile[:, dq * 2:(dq + 1) * 2],
                        mem_bf[mc4 * 4 + dq * 2:mc4 * 4 + (dq + 1) * 2]
                        .rearrange("c p d -> p c d"))
            for c in range(4):
                mc = mc4 * 4 + c
                tp = ps_trw.tile([P, B_CORE], F32, tag="wtr")
                for bt in range(NBT):
                    nc.tensor.transpose(
                        tp[:, bt * P:(bt + 1) * P],
                        w[bt][:, mc * P:(mc + 1) * P], ident[:])
                if mc % 2 == 0:
                    nc.vector.tensor_copy(wTt[:, mc], tp[:])
                else:
                    nc.scalar.copy(wTt[:, mc], tp[:])
                for bt in range(NBT):
                    for dh in range(2):
                        nc.tensor.matmul(
                            mv_ps[bt][dh][:],
                            wTt[:, mc, bt * P:(bt + 1) * P],
                            mtile[:, c, dh * 512:(dh + 1) * 512],
                            start=(mc == 0), stop=(mc == MC128 - 1))

        # evacuate mv to fp32 SBUF, transpose to mvT bf16 [P, dc, 256]
        mv_sb = [small.tile([P, D], F32, tag=f"mvsb{bt}", name=f"mvsb{bt}")
                 for bt in range(NBT)]
        for bt in range(NBT):
            for dh in range(2):
                nc.scalar.copy(mv_sb[bt][:, dh * 512:(dh + 1) * 512],
                               mv_ps[bt][dh][:])
        ps_mv.release()
        ps_trw.release()
        mvT = wpool.tile([P, DC, B_CORE], BF16, tag="w0", name="mvT")
        ps_mvt = tc.alloc_tile_pool(name="ps_mvt", bufs=4, space="PSUM")
        for dc in range(DC):
            tp = ps_mvt.tile([P, B_CORE], F32, tag="mvtr")
            for bt in range(NBT):
                nc.tensor.transpose(
                    tp[:, bt * P:(bt + 1) * P],
                    mv_sb[bt][:, dc * P:(dc + 1) * P], ident[:])
            nc.vector.tensor_copy(mvT[:, dc], tp[:])
        ps_mvt.release()

        # ---- MLP1: hT[hc] = relu(sum_dc W1-block^T @ mvT[dc] + b1[hc]) ----
        hT = wpool.tile([P, HC, B_CORE], BF16, tag="w1", name="hT")
        ps_h = tc.alloc_tile_pool(name="ps_h", bufs=4, space="PSUM")
        for hc in range(HC):
            hp = ps_h.tile([P, B_CORE], F32, tag="hp")
            for dc in range(DC):
                nc.tensor.matmul(
                    hp[:], w1t[:, dc, hc], mvT[:, dc],
                    start=(dc == 0), stop=(dc == DC - 1))
            nc.scalar.activation(
                hT[:, hc], hp[:], AF.Relu, bias=b1t[:, hc:hc + 1])
        ps_h.release()

        # ---- MLP2: out[bt] = hT-blocks^T @ W2 + b2 (nh outer, W2 slab DMA) ----
        ps_o = tc.alloc_tile_pool(name="ps_o", bufs=4, space="PSUM")
        osb = [small.tile([P, OUT], F32, tag=f"osb{bt}", name=f"osb{bt}")
               for bt in range(NBT)]
        NW = OUT // NH
        w2slabs = []
        for nh in range(NH):
            w2slab = mstream.tile([P, HC, NW], BF16, tag="memT",
                                  name=f"w2slab{nh}")
            for dq in range(2):
                nc.sync.dma_start(
                    w2slab[:, dq * 8:(dq + 1) * 8],
                    w2_bf[:, dq * 8:(dq + 1) * 8, nh * NW:(nh + 1) * NW])
            w2slabs.append(w2slab)
        for nh in range(NH):
            w2slab = w2slabs[nh]
            ops = [ps_o.tile([P, NW], F32, tag=f"op{bt}", name=f"op{bt}")
                   for bt in range(NBT)]
            for kc in range(HC):
                for bt in range(NBT):
                    nc.tensor.matmul(
                        ops[bt][:], hT[:, kc, bt * P:(bt + 1) * P],
                        w2slab[:, kc], start=(kc == 0), stop=False)
            for bt in range(NBT):
                nc.tensor.matmul(
                    ops[bt][:], ones1[:], b2t[:, nh * NW:(nh + 1) * NW],
                    start=False, stop=True)
                nc.scalar.copy(osb[bt][:, nh * NW:(nh + 1) * NW], ops[bt][:])
                nc.sync.dma_start(
                    out_d[bt, :, nh * NW:(nh + 1) * NW],
                    osb[bt][:, nh * NW:(nh + 1) * NW])
        ps_o.release()

        mstream.release()
        zpool.release()
        wpool.release()
        pers.release()
        small.release()

    nc.compile()
    return nc


_CACHED = None


def _prep(inputs):
    x = np.ascontiguousarray(inputs["encoder_output"], dtype=np.float32)
    mem = np.ascontiguousarray(inputs["memory_set"], dtype=np.float32)
    W1 = np.ascontiguousarray(inputs["W1"], dtype=np.float32)
    b1 = np.ascontiguousarray(inputs["b1"], dtype=np.float32)
    W2 = np.ascontiguousarray(inputs["W2"], dtype=np.float32)
    b2 = np.ascontiguousarray(inputs["b2"], dtype=np.float32)

    inv_ny = 1.0 / np.sqrt((mem * mem).sum(1) + _EPS)
    memT_hat = np.ascontiguousarray(
        (mem.T * inv_ny[None, :]).astype(np.float32).reshape(DC, P, M))
    mem_bfv = np.ascontiguousarray(mem.astype(bf16).reshape(MC128, P, D))
    # partition-major blocks: w1[p, dc, hc, c] = W1[dc*128+p, hc*128+c]
    w1_blk = np.ascontiguousarray(
        W1.astype(bf16).reshape(DC, P, HC, P).transpose(1, 0, 2, 3))
    # w2[p, kc, o] = W2[kc*128+p, o]
    w2_blk = np.ascontiguousarray(
        W2.astype(bf16).reshape(HC, P, OUT).transpose(1, 0, 2))
    b1_tiles = np.ascontiguousarray(b1.reshape(HC, P).T.astype(np.float32))
    b2_row = np.ascontiguousarray(b2.reshape(1, OUT).astype(np.float32))

    shared = {
        "memT": memT_hat, "mem_bf": mem_bfv, "w1_bf": w1_blk,
        "w2_bf": w2_blk, "b1_t": b1_tiles, "b2_r": b2_row,
    }
    in_maps = []
    for c in range(8):
        xs = np.ascontiguousarray(
            x[c * B_CORE:(c + 1) * B_CORE].reshape(NBT, P, D))
        in_maps.append({"x_s": xs, **shared})
    return in_maps


def kernel(**inputs) -> np.ndarray:
    global _CACHED
    if _CACHED is None:
        _CACHED = build()
    nc = _CACHED
    in_maps = _prep(inputs)
    res = run_bass_kernel_spmd(nc, in_maps, core_ids=list(range(8)))
    return np.concatenate(
        [r["out"].reshape(B_CORE, OUT) for r in res.results], axis=0)



# revision 6
# speedup vs baseline: 1.1603x; 1.1603x over previous
"""Trainium2 Bass kernel for nn_BaselineMemory (sparse attention memory read + MLP).

Data-parallel over batch: each of 8 NeuronCores handles 256 of 2048 rows.
Pipeline per core:
  x-norm (ACT/DVE) -> bf16 x_hat -> XBAR transpose -> dist matmul
  z = x_hat @ y_hat^T in bf16 (PE), evacuated as fp32 head + bf16 tail with
  fused per-chunk sum/max (+ Sum z^2 subsample for sigma)
  -> sparsemax tau via Gaussian-statistics init (tau0 = mu + a*.sigma,
     analytic Newton slope) + log-secant; 4 S-passes total
     (ACT relu+bias+accum on head, DVE max+add-accum on tail)
  -> w bf16 -> XBAR transpose -> memory read mv (bf16, PE)
  -> XBAR mv transpose -> MLP1 (bf16, b1 fused ACT bias + ReLU)
  -> MLP2 (bf16; b2 via rank-1 fp32r matmul) -> fp32 out.
"""
import sys

if "/opt/trn_rl_repo" not in sys.path:
    sys.path.insert(0, "/opt/trn_rl_repo")

import numpy as np
import ml_dtypes

import concourse.bass as bass  # noqa: F401
import concourse.tile as tile
from concourse import bacc, mybir
from concourse.bass_utils import run_bass_kernel_spmd

P = 128
B_CORE = 256          # batch rows per core
NBT = 2               # 2 b-tiles of 128
D = 1024
DC = D // P           # 8 d-chunks
M = 8192
MC512 = M // 512      # 16 m-chunks for dist
MC128 = M // P        # 64 m-chunks for read
H = 2048
HC = H // P           # 16 h-chunks
OUT = 1000
NH = 2                # out halves of 500
NW = OUT // NH

A_HEAD = 3072         # sparsemax: ACT handles m [0, A_HEAD)
AC = A_HEAD // 512    # 6 head chunks
TAIL = M - A_HEAD     # 5120 on DVE
NPASS = 4             # S-passes (last materializes w)
NSUB = 2048           # sigma estimated from first NSUB cols (4 chunks)
NSQC = NSUB // 512
ASTAR = 2.277844889   # Gaussian init: solves phi(a)-a*Q(a) = 1/(M*signom)
CK = 3.355671481e-4   # signom / (M*Q(astar)) : analytic 1/k = CK/sigma
CAP_OFF = 1e-4

F32 = mybir.dt.float32
F32R = mybir.dt.float32r
BF16 = mybir.dt.bfloat16
AF = mybir.ActivationFunctionType
ALU = mybir.AluOpType
AX = mybir.AxisListType
bf16 = ml_dtypes.bfloat16

_EPS = 1e-6
DEBUG = False


def build():
    nc = bacc.Bacc("TRN2", target_bir_lowering=False, debug=False)

    x_s = nc.dram_tensor("x_s", [NBT, P, D], F32, kind="ExternalInput")
    memT = nc.dram_tensor("memT", [DC, P, M], BF16, kind="ExternalInput")
    mem_bf = nc.dram_tensor("mem_bf", [MC128, P, D], BF16, kind="ExternalInput")
    w1_bf = nc.dram_tensor("w1_bf", [P, DC, HC, P], BF16, kind="ExternalInput")
    w2_bf = nc.dram_tensor("w2_bf", [P, HC, OUT], BF16, kind="ExternalInput")
    b1_t = nc.dram_tensor("b1_t", [P, HC], F32, kind="ExternalInput")
    b2_r = nc.dram_tensor("b2_r", [1, OUT], F32R, kind="ExternalInput")
    out_d = nc.dram_tensor("out", [NBT, P, OUT], F32, kind="ExternalOutput")
    if DEBUG:
        dbg_d = nc.dram_tensor("dbg", [16, P, NBT], F32, kind="ExternalOutput")

    with tile.TileContext(nc) as tc:
        small = tc.alloc_tile_pool(name="small", bufs=1)
        pers = tc.alloc_tile_pool(name="pers", bufs=1)

        eps_t = small.tile([P, 1], F32, tag="eps")
        nc.vector.memset(eps_t[:], _EPS)
        b1t = small.tile([P, HC], F32, tag="b1")
        nc.sync.dma_start(b1t[:], b1_t[:])
        b2t = small.tile([1, OUT], F32R, tag="b2")
        nc.sync.dma_start(b2t[:], b2_r[:])
        ones1f = small.tile([1, P], F32, tag="ones1f")
        nc.vector.memset(ones1f[:], 1.0)
        ones1 = small.tile([1, P], F32R, tag="ones1")
        nc.vector.tensor_copy(ones1[:], ones1f[:])

        # ---- x load + normalize -> bf16 -> XBAR transpose -> xnT ----
        xnT = pers.tile([P, DC, B_CORE], BF16, tag="xnT")
        xhat = [pers.tile([P, D], BF16, tag=f"xhat{bt}", name=f"xhat{bt}")
                for bt in range(NBT)]
        xpool = tc.alloc_tile_pool(name="xpool", bufs=1)
        for bt in range(NBT):
            xt = xpool.tile([P, D], F32, tag=f"x{bt}")
            nc.sync.dma_start(xt[:], x_s[bt])
            sq = xpool.tile([P, D], F32, tag="sqscr")
            ss = small.tile([P, 1], F32, tag=f"ss{bt}")
            nc.scalar.activation(sq[:], xt[:], AF.Square, accum_out=ss[:])
            nrm = small.tile([P, 1], F32, tag=f"nrm{bt}")
            nc.scalar.activation(nrm[:], ss[:], AF.Sqrt, bias=eps_t[:, 0:1])
            inv = small.tile([P, 1], F32, tag=f"inv{bt}")
            nc.vector.reciprocal(inv[:], nrm[:])
            nc.vector.tensor_scalar(
                out=xhat[bt][:], in0=xt[:], scalar1=inv[:, 0:1], scalar2=None,
                op0=ALU.mult)
            nc.sync.dma_start_transpose(
                out=xnT[:, :, bt * P:(bt + 1) * P], in_=xhat[bt][:])
        xpool.release()

        # ---- z storage: fp32 head + bf16 tail; bf16 scr; w bf16 ----
        zpool = tc.alloc_tile_pool(name="zpool", bufs=1)
        zh = [zpool.tile([P, A_HEAD], F32, tag=f"zh{bt}", name=f"zh{bt}")
              for bt in range(NBT)]
        zt_ = [zpool.tile([P, TAIL], BF16, tag=f"zt{bt}", name=f"zt{bt}")
               for bt in range(NBT)]
        scr = zpool.tile([P, TAIL], BF16, tag="scr0", name="scr0")
        zzero = zpool.tile([P, TAIL], BF16, tag="zzero", name="zzero")
        nc.vector.memset(zzero[:], 0.0)
        whs = zpool.tile([P, A_HEAD], BF16, tag="whs")   # head pass scratch
        sqz = zpool.tile([P, 512], F32, tag="sqz")       # ACT square scratch
        wpool = tc.alloc_tile_pool(name="wpool", bufs=1)
        wb = [wpool.tile([P, M], BF16, tag=f"w{bt}", name=f"w{bt}")
              for bt in range(NBT)]
        wTt = pers.tile([P, MC128, B_CORE], BF16, tag="wTt")
        mstream = tc.alloc_tile_pool(name="mstream", bufs=2)

        mx = [small.tile([P, MC512], F32, tag=f"mx{bt}", name=f"mx{bt}")
              for bt in range(NBT)]
        zsum = [small.tile([P, MC512], F32, tag=f"zs{bt}", name=f"zs{bt}")
                for bt in range(NBT)]
        zsq = [small.tile([P, NSQC], F32, tag=f"zq{bt}", name=f"zq{bt}")
               for bt in range(NBT)]

        # ---- dist matmul (bf16): z tiles + chunk sums/maxes + sum z^2 ----
        ps_dist = tc.alloc_tile_pool(name="ps_dist", bufs=6, space="PSUM")
        for mc in range(MC512):
            mtile = mstream.tile([P, DC, 512], BF16, tag="slab")
            for dq in range(4):
                nc.sync.dma_start(
                    mtile[:, dq * 2:(dq + 1) * 2],
                    memT[dq * 2:(dq + 1) * 2, :, mc * 512:(mc + 1) * 512]
                    .rearrange("d p m -> p d m"))
            for bt in range(NBT):
                zp = ps_dist.tile([P, 512], F32, tag="zp")
                for dc in range(DC):
                    nc.tensor.matmul(
                        zp[:], xnT[:, dc, bt * P:(bt + 1) * P], mtile[:, dc],
                        start=(dc == 0), stop=(dc == DC - 1))
                if mc < AC:
                    dst = zh[bt][:, mc * 512:(mc + 1) * 512]
                else:
                    dst = zt_[bt][:, (mc - AC) * 512:(mc - AC + 1) * 512]
                nc.vector.tensor_scalar(
                    out=dst, in0=zp[:], scalar1=0.0, scalar2=None,
                    op0=ALU.add, op1=ALU.add,
                    accum_out=zsum[bt][:, mc:mc + 1])
                nc.vector.reduce_max(mx[bt][:, mc:mc + 1], zp[:], axis=AX.X)
                if mc < NSQC:
                    nc.scalar.activation(
                        sqz[:], zp[:], AF.Square,
                        accum_out=zsq[bt][:, mc:mc + 1])
        ps_dist.release()

        # ---- sparsemax state ([P, 2]: one column per b-tile) ----
        def s2(nm):
            return small.tile([P, NBT], F32, tag=nm, name=nm)

        zst, rm2, msq = s2("zst"), s2("rm2"), s2("msq")
        mu, var, sig, rsig, kinv = s2("mu"), s2("var"), s2("sig"), s2("rsig"), s2("kinv")
        cap, tau_c, tau_p, ntau = s2("cap"), s2("tau_c"), s2("tau_p"), s2("ntau")
        sv, l_c, l_p, sact2, g2 = s2("sv"), s2("l_c"), s2("l_p"), s2("sact2"), s2("g2")
        stp, dl, dt, q_t, t2 = s2("stp"), s2("dl"), s2("dt"), s2("q_t"), s2("t2")

        for bt in range(NBT):
            nc.vector.reduce_sum(zst[:, bt:bt + 1], zsum[bt][:], axis=AX.X)
            nc.vector.reduce_max(rm2[:, bt:bt + 1], mx[bt][:], axis=AX.X)
            nc.vector.reduce_sum(msq[:, bt:bt + 1], zsq[bt][:], axis=AX.X)
        nc.vector.tensor_scalar_mul(mu[:], zst[:], 1.0 / M)
        nc.vector.tensor_scalar_add(cap[:], rm2[:], -CAP_OFF)
        nc.vector.tensor_scalar_mul(msq[:], msq[:], 1.0 / NSUB)
        nc.vector.tensor_tensor(var[:], mu[:], mu[:], ALU.mult)
        nc.vector.tensor_tensor(var[:], msq[:], var[:], ALU.subtract)
        nc.vector.tensor_scalar_max(var[:], var[:], 1e-12)
        nc.scalar.activation(sig[:], var[:], AF.Sqrt)
        nc.vector.reciprocal(rsig[:], sig[:])
        nc.vector.tensor_scalar_mul(kinv[:], rsig[:], CK)
        nc.vector.tensor_scalar_mul(tau_c[:], sig[:], ASTAR)
        nc.vector.tensor_tensor(tau_c[:], tau_c[:], mu[:], ALU.add)
        nc.vector.tensor_tensor(tau_c[:], tau_c[:], cap[:], ALU.min)
        nc.vector.tensor_scalar_mul(ntau[:], tau_c[:], -1.0)
        if DEBUG:
            for i, t in enumerate([mu, sig, kinv, cap, tau_c, zst, msq, rm2]):
                nc.sync.dma_start(dbg_d[i], t[:])

        # prefetch first read slabs during the sparsemax window
        rslabs = {}
        for i in range(2):
            sl = mstream.tile([P, 4, D], BF16, tag="slab", name=f"rslab{i}")
            for dq in range(2):
                nc.sync.dma_start(
                    sl[:, dq * 2:(dq + 1) * 2],
                    mem_bf[i * 4 + dq * 2:i * 4 + (dq + 1) * 2]
                    .rearrange("c p d -> p c d"))
            rslabs[i] = sl

        # ---- S-passes: ACT head (relu+bias+accum) + DVE tail (max+accum) ----
        ps_warm = tc.alloc_tile_pool(name="ps_warm", bufs=2, space="PSUM")
        for it in range(NPASS):
            last = (it == NPASS - 1)
            for bt in range(NBT):
                if not last:
                    nc.scalar.activation(
                        whs[:], zh[bt][:], AF.Relu, bias=ntau[:, bt:bt + 1],
                        accum_out=sact2[:, bt:bt + 1])
                    nc.vector.scalar_tensor_tensor(
                        out=scr[:], in0=zt_[bt][:],
                        scalar=tau_c[:, bt:bt + 1], in1=zzero[:],
                        op0=ALU.subtract, op1=ALU.max,
                        accum_out=g2[:, bt:bt + 1])
                else:
                    nc.scalar.activation(
                        wb[bt][:, 0:A_HEAD], zh[bt][:], AF.Relu,
                        bias=ntau[:, bt:bt + 1], accum_out=sact2[:, bt:bt + 1])
                    nc.vector.scalar_tensor_tensor(
                        out=wb[bt][:, A_HEAD:M], in0=zt_[bt][:],
                        scalar=tau_c[:, bt:bt + 1], in1=zzero[:],
                        op0=ALU.subtract, op1=ALU.max)
                wp = ps_warm.tile([P, 512], F32, tag="warm")
                nc.tensor.matmul(wp[:], xnT[:, 0, 0:P], xhat[0][:, 0:512],
                                 start=True, stop=True)
            if last:
                break
            # S = sact + g ; guard; ln
            nc.vector.tensor_tensor(sv[:], sact2[:], g2[:], ALU.add)
            nc.vector.tensor_scalar_max(sv[:], sv[:], 1e-30)
            nc.scalar.activation(l_c[:], sv[:], AF.Ln)
            if it == 0:
                # Newton with analytic Gaussian slope: step = (S-1)*kinv
                nc.vector.tensor_scalar_add(stp[:], sv[:], -1.0)
                nc.vector.tensor_tensor(stp[:], stp[:], kinv[:], ALU.mult)
            else:
                # log-secant: step = min(|dt|/|dl|, 2) * l_c
                nc.vector.tensor_tensor(dl[:], l_p[:], l_c[:], ALU.subtract)
                nc.vector.tensor_scalar_mul(t2[:], dl[:], -1.0)
                nc.vector.tensor_tensor(dl[:], dl[:], t2[:], ALU.max)
                nc.vector.tensor_scalar_max(dl[:], dl[:], 1e-12)
                nc.vector.reciprocal(dl[:], dl[:])
                nc.vector.tensor_tensor(dt[:], tau_c[:], tau_p[:], ALU.subtract)
                nc.vector.tensor_scalar_mul(t2[:], dt[:], -1.0)
                nc.vector.tensor_tensor(dt[:], dt[:], t2[:], ALU.max)
                nc.vector.tensor_tensor(q_t[:], dt[:], dl[:], ALU.mult)
                nc.vector.tensor_scalar_min(q_t[:], q_t[:], 2.0)
                nc.vector.tensor_tensor(stp[:], q_t[:], l_c[:], ALU.mult)
            nc.vector.tensor_copy(tau_p[:], tau_c[:])
            nc.vector.tensor_copy(l_p[:], l_c[:])
            nc.vector.tensor_tensor(tau_c[:], tau_c[:], stp[:], ALU.add)
            nc.vector.tensor_tensor(tau_c[:], tau_c[:], cap[:], ALU.min)
            nc.vector.tensor_scalar_mul(ntau[:], tau_c[:], -1.0)
            if DEBUG:
                nc.sync.dma_start(dbg_d[8 + it], tau_c[:])
                nc.sync.dma_start(dbg_d[11 + it], sv[:])
                if it == 0:
                    nc.sync.dma_start(dbg_d[14], sact2[:])
                    nc.sync.dma_start(dbg_d[15], g2[:])
        ps_warm.release()

        # ---- w^T via XBAR (quartered for pipelining with the read) ----
        for q in range(4):
            for bt in range(NBT):
                nc.sync.dma_start_transpose(
                    out=wTt[:, q * 16:(q + 1) * 16, bt * P:(bt + 1) * P],
                    in_=wb[bt][:, q * 2048:(q + 1) * 2048])

        # ---- read: mv[bt] += wT-chunk @ mem-chunk over 64 m-chunks ----
        ps_mv = tc.alloc_tile_pool(name="ps_mv", bufs=1, space="PSUM")
        mv_ps = [[ps_mv.tile([P, 512], F32, tag=f"mv{bt}_{dh}", name=f"mv{bt}_{dh}")
                  for dh in range(2)] for bt in range(NBT)]
        for mc4 in range(MC128 // 4):
            nxt = mc4 + 2
            if nxt < MC128 // 4:
                sl = mstream.tile([P, 4, D], BF16, tag="slab", name=f"rslab{nxt}")
                for dq in range(2):
                    nc.sync.dma_start(
                        sl[:, dq * 2:(dq + 1) * 2],
                        mem_bf[nxt * 4 + dq * 2:nxt * 4 + (dq + 1) * 2]
                        .rearrange("c p d -> p c d"))
                rslabs[nxt] = sl
            mtile = rslabs.pop(mc4)
            for c in range(4):
                mc = mc4 * 4 + c
                for bt in range(NBT):
                    for dh in range(2):
                        nc.tensor.matmul(
                            mv_ps[bt][dh][:],
                            wTt[:, mc, bt * P:(bt + 1) * P],
                            mtile[:, c, dh * 512:(dh + 1) * 512],
                            start=(mc == 0), stop=(mc == MC128 - 1))

        # stream W1 quarters + W2 slabs into the freed z/scr tag slots
        w1q = []
        for qh in range(4):
            tg = [f"zh0", f"zh1", f"zt0", f"zt1"][qh]
            t = zpool.tile([P, DC, 4, P], BF16, tag=tg, name=f"w1q{qh}")
            nc.sync.dma_start(t[:], w1_bf[:, :, qh * 4:(qh + 1) * 4])
            w1q.append(t)
        w2slabs = []
        for nh in range(NH):
            tg = ["scr0", "whs"][nh]
            t = zpool.tile([P, HC, NW], BF16, tag=tg, name=f"w2s{nh}")
            for dq in range(2):
                nc.sync.dma_start(
                    t[:, dq * 8:(dq + 1) * 8],
                    w2_bf[:, dq * 8:(dq + 1) * 8, nh * NW:(nh + 1) * NW])
            w2slabs.append(t)

        # evacuate mv -> bf16, XBAR-transpose to mvT [P, dc, 256]
        mv_sb = [pers.tile([P, D], BF16, tag=f"mvsb{bt}", name=f"mvsb{bt}")
                 for bt in range(NBT)]
        mvT = pers.tile([P, DC, B_CORE], BF16, tag="mvT")
        for bt in range(NBT):
            for dh in range(2):
                nc.scalar.copy(mv_sb[bt][:, dh * 512:(dh + 1) * 512],
                               mv_ps[bt][dh][:])
            nc.sync.dma_start_transpose(
                out=mvT[:, :, bt * P:(bt + 1) * P], in_=mv_sb[bt][:])
        ps_mv.release()

        # ---- MLP1: hT[hc] = relu(sum_dc W1-block^T @ mvT[dc] + b1[hc]) ----
        hT = pers.tile([P, HC, B_CORE], BF16, tag="hT")
        ps_h = tc.alloc_tile_pool(name="ps_h", bufs=4, space="PSUM")
        for hc in range(HC):
            hp = ps_h.tile([P, B_CORE], F32, tag="hp")
            for dc in range(DC):
                nc.tensor.matmul(
                    hp[:], w1q[hc // 4][:, dc, hc % 4], mvT[:, dc],
                    start=(dc == 0), stop=(dc == DC - 1))
            nc.scalar.activation(
                hT[:, hc], hp[:], AF.Relu, bias=b1t[:, hc:hc + 1])
        ps_h.release()

        # ---- MLP2: out[bt] = hT-blocks^T @ W2 + b2 ----
        ps_o = tc.alloc_tile_pool(name="ps_o", bufs=4, space="PSUM")
        osb = [pers.tile([P, OUT], F32, tag=f"osb{bt}", name=f"osb{bt}")
               for bt in range(NBT)]
        for nh in range(NH):
            w2slab = w2slabs[nh]
            ops = [ps_o.tile([P, NW], F32, tag=f"op{bt}", name=f"op{bt}")
                   for bt in range(NBT)]
            for kc in range(HC):
                for bt in range(NBT):
                    nc.tensor.matmul(
                        ops[bt][:], hT[:, kc, bt * P:(bt + 1) * P],
                        w2slab[:, kc], start=(kc == 0), stop=False)
            for bt in range(NBT):
                nc.tensor.matmul(
                    ops[bt][:], ones1[:], b2t[:, nh * NW:(nh + 1) * NW],
                    start=False, stop=True)
                nc.scalar.copy(osb[bt][:, nh * NW:(nh + 1) * NW], ops[bt][:])
                nc.sync.dma_start(
                    out_d[bt, :, nh * NW:(nh + 1) * NW],
                    osb[bt][:, nh * NW:(nh + 1) * NW])
        ps_o.release()

        mstream.release()
        wpool.release()
        zpool.release()
        pers.release()
        small.release()

    nc.compile()
    return nc


_CACHED = None


def _prep(inputs):
    x = np.ascontiguousarray(inputs["encoder_output"], dtype=np.float32)
    mem = np.ascontiguousarray(inputs["memory_set"], dtype=np.float32)
    W1 = np.ascontiguousarray(inputs["W1"], dtype=np.float32)
    b1 = np.ascontiguousarray(inputs["b1"], dtype=np.float32)
    W2 = np.ascontiguousarray(inputs["W2"], dtype=np.float32)
    b2 = np.ascontiguousarray(inputs["b2"], dtype=np.float32)

    inv_ny = 1.0 / np.sqrt((mem * mem).sum(1) + _EPS)
    memT_hat = np.ascontiguousarray(
        (mem.T * inv_ny[None, :]).astype(bf16).reshape(DC, P, M))
    mem_bfv = np.ascontiguousarray(mem.astype(bf16).reshape(MC128, P, D))
    # partition-major blocks: w1[p, dc, hc, c] = W1[dc*128+p, hc*128+c]
    w1_blk = np.ascontiguousarray(
        W1.astype(bf16).reshape(DC, P, HC, P).transpose(1, 0, 2, 3))
    # w2[p, kc, o] = W2[kc*128+p, o]
    w2_blk = np.ascontiguousarray(
        W2.astype(bf16).reshape(HC, P, OUT).transpose(1, 0, 2))
    b1_tiles = np.ascontiguousarray(b1.reshape(HC, P).T.astype(np.float32))
    b2_row = np.ascontiguousarray(b2.reshape(1, OUT).astype(np.float32))

    shared = {
        "memT": memT_hat, "mem_bf": mem_bfv, "w1_bf": w1_blk,
        "w2_bf": w2_blk, "b1_t": b1_tiles, "b2_r": b2_row,
    }
    in_maps = []
    for c in range(8):
        xs = np.ascontiguousarray(
            x[c * B_CORE:(c + 1) * B_CORE].reshape(NBT, P, D))
        in_maps.append({"x_s": xs, **shared})
    return in_maps


def kernel(**inputs) -> np.ndarray:
    global _CACHED
    if _CACHED is None:
        _CACHED = build()
    nc = _CACHED
    in_maps = _prep(inputs)
    res = run_bass_kernel_spmd(nc, in_maps, core_ids=list(range(8)))
    return np.concatenate(
        [r["out"].reshape(B_CORE, OUT) for r in res.results], axis=0)
